# revision 1
# baseline (speedup 1.0000x reference)
"""Trainium2 Bass kernel for nn_FFMLP (4-layer MLP, hidden=128, relu).

Strategy (pure data parallel, batch sharded 8 ways):
- Feature-major on-chip layout: activations live as [feat, batch] so every
  layer is a single K<=128 matmul with the (tiny, replicated) weight as the
  stationary operand and the activation stream as the moving operand.
- fp16 matmul operands (1 cycle/row on the PE vs 4 for fp32), fp32 PSUM
  accumulation; host rounds inputs/weights to fp16 (unbiased ~2^-11).
- Per-512-column chunk pipeline: L0..L3 outputs each occupy one PSUM bank;
  ReLU + downcast PSUM->SBUF is split between ScalarE and VectorE (the
  structural bottleneck: ~1 elem/cycle/lane each from PSUM).
- L4 (M=16) is packed 4 chunks deep into one PSUM bank via column tiling
  (tile_position=(0,32j)) so the final fp32 evacuation is amortized 4x.
- Host transposes x -> x.T per shard and the [16, B/8] result back.
"""
import sys

if "/opt/trn_rl_repo" not in sys.path:
    sys.path.insert(0, "/opt/trn_rl_repo")

import numpy as np

import concourse.bass as bass
import concourse.mybir as mybir
import concourse.tile as tile

INPUT_DIM = 32
OUTPUT_DIM = 16
HIDDEN = 128
PADDED_OUT = 16
NUM_LAYERS = 4
B = 524288
N_CORES = 8
B_CORE = B // N_CORES  # 65536
CHUNK = 512
N_CHUNKS = B_CORE // CHUNK  # 128
GROUP = 4  # chunks packed per L4 PSUM bank (column tiling)
IN_SLAB = 8  # chunks per input DMA

fp16 = mybir.dt.float16
fp32 = mybir.dt.float32
RELU = mybir.ActivationFunctionType.Relu


def _split_waits(nc, max_waits=1):
    """walrus in this image rejects >1 semaphore wait per instruction on some
    formats; split excess waits onto preceding NOPs on the same engine queue
    (queues are in-order, so semantics are preserved)."""
    n_new = 0
    for bb in nc.main_func.blocks:
        out_list = []
        changed = False
        for ins in bb.instructions:
            si = ins.sync_info
            if si is not None and si.on_wait and len(si.on_wait) > max_waits:
                waits = list(si.on_wait)
                extra, keep = waits[:-max_waits], waits[-max_waits:]
                while extra:
                    chunk, extra = extra[:max_waits], extra[max_waits:]
                    n_new += 1
                    nop = mybir.InstNoOp(name=f"I-waitsplit-{n_new}", ins=[], outs=[])
                    nop.engine = ins.engine
                    nop.sync_info = mybir.SyncInfo(on_wait=chunk, on_update=[])
                    out_list.append(nop)
                ins.sync_info = mybir.SyncInfo(on_wait=keep, on_update=si.on_update)
                changed = True
            out_list.append(ins)
        if changed:
            bb.instructions = out_list
    return n_new


def _dedup_ldweights(nc):
    """Tile emits an explicit InstLdweights before every matmul; weights only
    change at those instructions. Replace an InstLdweights whose key
    (weights AP, tile position/size, perf mode) matches the previous one on
    the PE queue with a NOP carrying the same sync_info — the weight reload
    otherwise costs ~93ns ahead of its matmul."""
    n = 0
    for bb in nc.main_func.blocks:
        il = list(bb.instructions)
        last_key = None
        changed = False
        for idx, ins in enumerate(il):
            if ins.engine != mybir.EngineType.PE:
                continue
            if isinstance(ins, mybir.InstLdweights):
                key = (
                    repr(ins.ins[0]),
                    str(ins.tile_position),
                    str(getattr(ins, "tile_size", None)),
                    str(ins.perf_mode),
                    bool(ins.is_transpose),
                )
                if key == last_key:
                    nop = mybir.InstNoOp(name=ins.name, ins=[], outs=[])
                    nop.engine = ins.engine
                    nop.sync_info = ins.sync_info
                    il[idx] = nop
                    changed = True
                    n += 1
                last_key = key
        if changed:
            bb.instructions = il
    return n


def build(n_chunks=N_CHUNKS):
    nc = bass.Bass()
    n_cols = n_chunks * CHUNK
    # xt2: pair-strip layout — xt2[32*i + f, p*CHUNK + c] = x.T[f, (2p+i)*CHUNK + c]
    # so a pair of chunks feeds two concurrent row-tiled K=32 L0 matmuls.
    xt = nc.declare_dram_parameter(
        "xt", [2 * INPUT_DIM, n_cols // 2], fp16, isOutput=False
    )
    w0 = nc.declare_dram_parameter(
        "w0", [2 * INPUT_DIM, HIDDEN], fp16, isOutput=False
    )
    w1 = nc.declare_dram_parameter("w1", [HIDDEN, HIDDEN], fp16, isOutput=False)
    w2 = nc.declare_dram_parameter("w2", [HIDDEN, HIDDEN], fp16, isOutput=False)
    w3 = nc.declare_dram_parameter("w3", [HIDDEN, HIDDEN], fp16, isOutput=False)
    w4 = nc.declare_dram_parameter("w4", [HIDDEN, PADDED_OUT], fp16, isOutput=False)
    yt = nc.declare_dram_parameter("yt", [PADDED_OUT, n_cols], fp32, isOutput=True)

    with tile.TileContext(nc) as tc:
        with (
            tc.tile_pool(name="wp", bufs=1) as wp,
            tc.tile_pool(name="io", bufs=1) as io,
            tc.tile_pool(name="hp", bufs=1) as hp,
            tc.tile_pool(name="ps", bufs=1, space="PSUM") as ps,
        ):
            w0s = wp.tile([2 * INPUT_DIM, HIDDEN], fp16, tag="w0", name="w0s")
            w1s = wp.tile([HIDDEN, HIDDEN], fp16, tag="w1", name="w1s")
            w2s = wp.tile([HIDDEN, HIDDEN], fp16, tag="w2", name="w2s")
            w3s = wp.tile([HIDDEN, HIDDEN], fp16, tag="w3", name="w3s")
            w4s = wp.tile([HIDDEN, PADDED_OUT], fp16, tag="w4", name="w4s")
            nc.sync.dma_start(out=w0s, in_=w0[:, :])
            nc.sync.dma_start(out=w1s, in_=w1[:, :])
            nc.sync.dma_start(out=w2s, in_=w2[:, :])
            nc.sync.dma_start(out=w3s, in_=w3[:, :])
            nc.sync.dma_start(out=w4s, in_=w4[:, :])

            # Software-pipelined emission over chunk pairs. Per round, the
            # deepest stages are emitted first so adjacent PE-queue matmuls
            # come from different stages/chunks and can stream back-to-back.
            #   stage0(pair p)  @ round 2p  : 2 row-tiled L0 MMs -> l0 pair
            #                                 tile, ACT relu FD=1024 -> h1
            #   stage1(chunk c) @ round c+1 : L1 MM, DVE relu -> h2
            #   stage2(pair p)  @ round 2p+3: 2 L2 MMs -> l0 tile (reuse),
            #                                 ACT relu FD=1024 -> h3
            #   stage3(chunk c) @ round c+4 : L3 MM, DVE relu -> h4
            #   stage4(group g) @ round 4g+8: 4 adjacent col-tiled L4 MMs
            #                                 (concurrent), ACT evac, DMA out
            state = {}  # tiles carried between stages
            PAIR = 2 * CHUNK
            SLAB_PAIRS = IN_SLAB // 2  # pairs per input DMA

            # HAM warm-up: dummy matmuls keep the PE busy while the first
            # input slab lands, so real matmuls start at 2.4 GHz instead of
            # paying the ~3.4us cold window at 1.2 GHz.
            pwarm = ps.tile([HIDDEN, 128], fp32, tag="l4", bufs=1, name="pwarm")
            for _ in range(24):
                nc.tensor.matmul(
                    pwarm[:, :], w1s[:, :], w2s[:, 0:128], start=True, stop=True
                )

            def stage0(p):
                if p % SLAB_PAIRS == 0:
                    npair = min(SLAB_PAIRS, n_chunks // 2 - p)
                    state["xslab", p // SLAB_PAIRS] = xs = io.tile(
                        [2 * INPUT_DIM, npair * CHUNK], fp16,
                        tag="xin", bufs=4, name="xs",
                    )
                    nc.sync.dma_start(
                        out=xs, in_=xt[:, p * CHUNK : (p + npair) * CHUNK]
                    )
                xs = state["xslab", p // SLAB_PAIRS]
                o = (p % SLAB_PAIRS) * CHUNK
                p0 = ps.tile([HIDDEN, PAIR], fp32, tag="l0", bufs=2, name="p0")
                for i in range(2):
                    nc.tensor.matmul(
                        p0[:, i * CHUNK : (i + 1) * CHUNK],
                        w0s[32 * i : 32 * i + INPUT_DIM, :],
                        xs[32 * i : 32 * i + INPUT_DIM, o : o + CHUNK],
                        start=True,
                        stop=True,
                        tile_position=(32 * i, 0),
                    )
                h1 = hp.tile([HIDDEN, PAIR], fp16, tag="h1", bufs=2, name="h1")
                nc.scalar.activation(h1[:, :], p0[:, :], RELU)
                state["h1", p] = h1
                state["p0", p] = p0

            def stage1(c):
                p = c // 2
                h1 = state[("h1", p)]
                i = c % 2
                p1 = ps.tile([HIDDEN, CHUNK], fp32, tag="l1", bufs=2, name="p1")
                nc.tensor.matmul(
                    p1[:, :], w1s[:, :], h1[:, i * CHUNK : (i + 1) * CHUNK],
                    start=True, stop=True,
                )
                if i == 1:
                    del state[("h1", p)]
                h2 = hp.tile([HIDDEN, CHUNK], fp16, tag="h2", bufs=6, name="h2")
                nc.vector.tensor_scalar_max(h2[:, :], p1[:, :], 0.0)
                state["h2", c] = h2

            def stage2(p):
                p0 = state.pop(("p0", p))
                h3 = hp.tile([HIDDEN, PAIR], fp16, tag="h3", bufs=2, name="h3")
                for i in range(2):
                    h2 = state.pop(("h2", 2 * p + i))
                    nc.tensor.matmul(
                        p0[:, i * CHUNK : (i + 1) * CHUNK],
                        w2s[:, :], h2[:, :],
                        start=True, stop=True,
                    )
                nc.scalar.activation(h3[:, :], p0[:, :], RELU)
                state["h3", p] = h3

            def stage3(c):
                p = c // 2
                h3 = state[("h3", p)]
                i = c % 2
                p3 = ps.tile([HIDDEN, CHUNK], fp32, tag="l3", bufs=1, name="p3")
                nc.tensor.matmul(
                    p3[:, :], w3s[:, :], h3[:, i * CHUNK : (i + 1) * CHUNK],
                    start=True, stop=True,
                )
                if i == 1:
                    del state[("h3", p)]
                h4 = hp.tile([HIDDEN, CHUNK], fp16, tag="h4", bufs=8, name="h4")
                nc.vector.tensor_scalar_max(h4[:, :], p3[:, :], 0.0)
                state["h4", c] = h4

            def stage4(g):
                p4 = ps.tile([HIDDEN, CHUNK], fp32, tag="l4", bufs=1, name="p4")
                for j in range(GROUP):
                    h4 = state.pop(("h4", 4 * g + j))
                    nc.tensor.matmul(
                        p4[32 * j : 32 * j + PADDED_OUT, :],
                        w4s[:, :],
                        h4[:, :],
                        start=True,
                        stop=True,
                        tile_position=(0, 32 * j),
                    )
                osb = io.tile([HIDDEN, CHUNK], fp32, tag="osb", bufs=4, name="osb")
                nc.scalar.copy(out=osb[:, :], in_=p4[:, :])
                for jj in range(GROUP):
                    cc = 4 * g + jj
                    nc.sync.dma_start(
                        out=yt[:, cc * CHUNK : (cc + 1) * CHUNK],
                        in_=osb[32 * jj : 32 * jj + PADDED_OUT, :],
                    )

            assert n_chunks % 4 == 0
            for r in range(n_chunks + 9):
                if r >= 8 and (r - 8) % 4 == 0 and (r - 8) // 4 < n_chunks // 4:
                    stage4((r - 8) // 4)
                if 0 <= r - 4 < n_chunks:
                    stage3(r - 4)
                if r >= 3 and (r - 3) % 2 == 0 and (r - 3) // 2 < n_chunks // 2:
                    stage2((r - 3) // 2)
                if 0 <= r - 1 < n_chunks:
                    stage1(r - 1)
                if r % 2 == 0 and r // 2 < n_chunks // 2:
                    stage0(r // 2)
    _dedup_ldweights(nc)
    _split_waits(nc)
    return nc


def _split_weights(weights):
    ws = []
    off = 0
    ws.append(weights[off : off + HIDDEN * INPUT_DIM].reshape(HIDDEN, INPUT_DIM))
    off += HIDDEN * INPUT_DIM
    for _ in range(NUM_LAYERS - 1):
        ws.append(weights[off : off + HIDDEN * HIDDEN].reshape(HIDDEN, HIDDEN))
        off += HIDDEN * HIDDEN
    ws.append(weights[off : off + PADDED_OUT * HIDDEN].reshape(PADDED_OUT, HIDDEN))
    return ws


_NC_CACHE = {}


def make_in_maps(inputs: np.ndarray, weights: np.ndarray):
    ws = _split_weights(np.asarray(weights, dtype=np.float32))
    # stationary operands are lhsT = [K_in, M_out] = W.T; W0.T is stacked
    # twice for the two row-tiled strips.
    w0t = np.ascontiguousarray(ws[0].T).astype(np.float16)
    wmaps = {
        "w0": np.concatenate([w0t, w0t], axis=0),
        "w1": np.ascontiguousarray(ws[1].T).astype(np.float16),
        "w2": np.ascontiguousarray(ws[2].T).astype(np.float16),
        "w3": np.ascontiguousarray(ws[3].T).astype(np.float16),
        "w4": np.ascontiguousarray(ws[4].T).astype(np.float16),
    }
    in_maps = []
    for i in range(N_CORES):
        xc = inputs[i * B_CORE : (i + 1) * B_CORE]
        xtc = np.ascontiguousarray(xc.T).astype(np.float16)  # [32, B_CORE]
        # pair-strip layout: [64, B_CORE//2]
        xt2 = np.ascontiguousarray(
            xtc.reshape(INPUT_DIM, B_CORE // (2 * CHUNK), 2, CHUNK)
            .transpose(2, 0, 1, 3)
            .reshape(2 * INPUT_DIM, B_CORE // 2)
        )
        in_maps.append({"xt": xt2, **wmaps})
    return in_maps


def kernel(inputs: np.ndarray, weights: np.ndarray) -> np.ndarray:
    from concourse.bass_utils import run_bass_kernel_spmd

    assert inputs.shape == (B, INPUT_DIM), inputs.shape
    in_maps = make_in_maps(inputs, weights)
    if "nc" not in _NC_CACHE:
        _NC_CACHE["nc"] = build()
    nc = _NC_CACHE["nc"]
    res = run_bass_kernel_spmd(nc, in_maps, list(range(N_CORES)))
    outs = [np.ascontiguousarray(r["yt"].T) for r in res.results]
    return np.concatenate(outs, axis=0)[:, :OUTPUT_DIM]



# revision 4
# speedup vs baseline: 1.2004x; 1.2004x over previous
"""Trainium2 Bass kernel for nn_FFMLP (4-layer MLP, hidden=128, relu).

Strategy (pure data parallel, batch sharded 8 ways):
- Feature-major on-chip layout: activations live as [feat, batch]; each layer
  is a K<=128 matmul with the tiny replicated weight stationary and the
  activation stream moving. fp16 operands, fp32 PSUM.
- Layer-batched slab schedule: per slab of 32 chunks, run all of L0, then all
  of L1, ... so consecutive PE matmuls share weights (one LdWeights per layer
  per slab instead of one per matmul -- the interleaved baseline paid ~50us
  of LDWEIGHTS on the PE queue).
- L0 (K=32) packs 4 chunks into concurrent row-tiled matmuls at
  tile_position (32i, 0); L4 (M=16) packs 4 chunks per PSUM bank via column
  tiling (0, 32j). Tiled matmuls execute concurrently on the PE.
- PSUM: 8 banks as two double-buffered 2-bank groups, one owned by the
  Scalar (ACT) engine, one by Vector (DVE). PSUM->SBUF relu evacuation is
  the structural bottleneck (only these two engines can read PSUM);
  a 6:5 ACT:DVE group pattern balances their 0.83 vs 1.10 ns/col rates and
  keeps both ~100% busy without bank starvation.
- Output is packed fp16 in a (group, bank, strip) lexicographic layout so the
  host just reshapes; final cast to fp32 on host.
"""
import sys

if "/opt/trn_rl_repo" not in sys.path:
    sys.path.insert(0, "/opt/trn_rl_repo")

import numpy as np

import concourse.bass as bass
import concourse.mybir as mybir
import concourse.tile as tile

INPUT_DIM = 32
OUTPUT_DIM = 16
HIDDEN = 128
PADDED_OUT = 16
NUM_LAYERS = 4
B = 524288
N_CORES = 8
B_CORE = B // N_CORES  # 65536
CHUNK = 512
N_CHUNKS = B_CORE // CHUNK  # 128
SLAB = 32  # chunks per slab (layer-batched unit)
N_SLABS = N_CHUNKS // SLAB  # 4
QUADS_PER_SLAB = SLAB // 4  # 8 (4-chunk quads for L0 row tiling)
GROUPS_PER_SLAB = SLAB // 2  # 16 2-chunk PSUM groups per layer phase
L4_GROUPS_PER_SLAB = SLAB // 8  # 4 (8 chunks of output per 2-bank group)
N_L4_GROUPS = N_CHUNKS // 8  # 16

fp16 = mybir.dt.float16
fp32 = mybir.dt.float32
RELU = mybir.ActivationFunctionType.Relu

# evac engine pattern: 6 ACT : 5 DVE matches 1/1038 : 1/1247 drain rates
EVAC_PAT = "ADADADADADA"


def _split_waits(nc, max_waits=1):
    """walrus in this image rejects >1 semaphore wait per instruction on some
    formats; split excess waits onto preceding NOPs on the same engine queue
    (queues are in-order, so semantics are preserved)."""
    n_new = 0
    for bb in nc.main_func.blocks:
        out_list = []
        changed = False
        for ins in bb.instructions:
            si = ins.sync_info
            if si is not None and si.on_wait and len(si.on_wait) > max_waits:
                waits = list(si.on_wait)
                extra, keep = waits[:-max_waits], waits[-max_waits:]
                while extra:
                    chunk, extra = extra[:max_waits], extra[max_waits:]
                    n_new += 1
                    nop = mybir.InstNoOp(name=f"I-waitsplit-{n_new}", ins=[], outs=[])
                    nop.engine = ins.engine
                    nop.sync_info = mybir.SyncInfo(on_wait=chunk, on_update=[])
                    out_list.append(nop)
                ins.sync_info = mybir.SyncInfo(on_wait=keep, on_update=si.on_update)
                changed = True
            out_list.append(ins)
        if changed:
            bb.instructions = out_list
    return n_new


def _ldw_rect(ins):
    """PE-array rectangle (r0, r1, c0, c1) occupied by an InstLdweights."""
    tp = ins.tile_position
    ts = getattr(ins, "tile_size", None)
    r0, c0 = (tp if tp else (0, 0))
    if ts:
        rows, cols = ts
    else:
        rows, cols = 128, 128
    return (r0, r0 + rows, c0, c0 + cols)


def _dedup_ldweights(nc):
    """Tile emits an explicit InstLdweights before every matmul. Weights at a
    given tile rectangle stay resident until an overlapping load clobbers
    them, so replace reloads of already-resident weights with NOPs (keeping
    sync_info). Tracks residency per array rectangle, which handles the
    alternating tile positions of the row/col-tiled L0/L4 phases."""
    n = 0
    for bb in nc.main_func.blocks:
        il = list(bb.instructions)
        live = {}  # rect -> content key
        changed = False
        for idx, ins in enumerate(il):
            if ins.engine != mybir.EngineType.PE:
                continue
            if isinstance(ins, mybir.InstLdweights):
                rect = _ldw_rect(ins)
                key = (
                    repr(ins.ins[0]),
                    str(ins.tile_position),
                    str(getattr(ins, "tile_size", None)),
                    str(ins.perf_mode),
                    bool(ins.is_transpose),
                )
                if live.get(rect) == key:
                    nop = mybir.InstNoOp(name=ins.name, ins=[], outs=[])
                    nop.engine = ins.engine
                    nop.sync_info = ins.sync_info
                    il[idx] = nop
                    changed = True
                    n += 1
                else:
                    r0, r1, c0, c1 = rect
                    for other in list(live):
                        o0, o1, p0, p1 = other
                        if r0 < o1 and o0 < r1 and c0 < p1 and p0 < c1:
                            del live[other]
                    live[rect] = key
        if changed:
            bb.instructions = il
    return n


def build(n_slabs=N_SLABS):
    nc = bass.Bass()
    n_chunks = n_slabs * SLAB
    # xt4: quad-strip layout -- xt4[32*i + f, q*CHUNK + c] = x.T[f, (4q+i)*CHUNK + c]
    # so each quad of 4 chunks feeds 4 concurrent row-tiled K=32 L0 matmuls.
    xt = nc.declare_dram_parameter(
        "xt", [4 * INPUT_DIM, n_chunks * CHUNK // 4], fp16, isOutput=False
    )
    w0 = nc.declare_dram_parameter("w0", [4 * INPUT_DIM, HIDDEN], fp16, isOutput=False)
    w1 = nc.declare_dram_parameter("w1", [HIDDEN, HIDDEN], fp16, isOutput=False)
    w2 = nc.declare_dram_parameter("w2", [HIDDEN, HIDDEN], fp16, isOutput=False)
    w3 = nc.declare_dram_parameter("w3", [HIDDEN, HIDDEN], fp16, isOutput=False)
    w4 = nc.declare_dram_parameter("w4", [HIDDEN, PADDED_OUT], fp16, isOutput=False)
    # yt[o, g, b, j, c] = y.T[o, ((8g + 4b + j)*CHUNK + c] -- lexicographic in
    # (g, b, j, c), so the host reshapes to [16, B_CORE] with no permute.
    n_l4_groups = n_chunks // 8
    yt = nc.declare_dram_parameter(
        "yt", [PADDED_OUT, n_l4_groups, 2, 4, CHUNK], fp16, isOutput=True
    )

    with tile.TileContext(nc) as tc:
        with (
            tc.tile_pool(name="wp", bufs=1) as wp,
            tc.tile_pool(name="io", bufs=1) as io,
            tc.tile_pool(name="hp", bufs=1) as hp,
            tc.tile_pool(name="ps", bufs=1, space="PSUM") as ps,
        ):
            w0s = wp.tile([4 * INPUT_DIM, HIDDEN], fp16, tag="w0", name="w0s")
            w1s = wp.tile([HIDDEN, HIDDEN], fp16, tag="w1", name="w1s")
            w2s = wp.tile([HIDDEN, HIDDEN], fp16, tag="w2", name="w2s")
            w3s = wp.tile([HIDDEN, HIDDEN], fp16, tag="w3", name="w3s")
            w4s = wp.tile([HIDDEN, PADDED_OUT], fp16, tag="w4", name="w4s")
            nc.sync.dma_start(out=w0s, in_=w0[:, :])
            nc.sync.dma_start(out=w1s, in_=w1[:, :])
            nc.sync.dma_start(out=w2s, in_=w2[:, :])
            nc.sync.dma_start(out=w3s, in_=w3[:, :])
            nc.sync.dma_start(out=w4s, in_=w4[:, :])

            # hidden activation ping-pong buffers, one slab each
            hA = hp.tile([HIDDEN, SLAB * CHUNK], fp16, tag="hA", name="hA")
            hB = hp.tile([HIDDEN, SLAB * CHUNK], fp16, tag="hB", name="hB")

            SLAB_COLS = SLAB * CHUNK // 4  # xt cols per slab (quad-packed)

            xs_tiles = {}

            def fetch_slab(s):
                if s >= n_slabs:
                    return
                xs = io.tile(
                    [4 * INPUT_DIM, SLAB_COLS], fp16, tag="xin", bufs=2, name="xs"
                )
                nc.sync.dma_start(out=xs, in_=xt[:, s * SLAB_COLS : (s + 1) * SLAB_COLS])
                xs_tiles[s] = xs

            # HAM warm-up: dummy matmuls keep the PE busy while the first
            # input slab lands, so real matmuls start at 2.4 GHz instead of
            # paying the ~3.4us cold window at 1.2 GHz.
            fetch_slab(0)
            pwarm = ps.tile([HIDDEN, 2 * CHUNK], fp32, tag="pA", bufs=2, name="pwarm")
            for _ in range(24):
                nc.tensor.matmul(
                    pwarm[:, 0:HIDDEN], w1s[:, :], w2s[:, 0:HIDDEN],
                    start=True, stop=True,
                )
            fetch_slab(1)

            evac_state = {"i": 0}

            def evac(dst, src, relu):
                eng = EVAC_PAT[evac_state["i"] % len(EVAC_PAT)]
                evac_state["i"] += 1
                if eng == "A":
                    if relu:
                        nc.scalar.activation(dst, src, RELU)
                    else:
                        nc.scalar.copy(out=dst, in_=src)
                else:
                    if relu:
                        nc.vector.tensor_scalar_max(dst, src, 0.0)
                    else:
                        nc.vector.tensor_copy(dst, src)

            def psum_group():
                eng = EVAC_PAT[evac_state["i"] % len(EVAC_PAT)]
                tag = "pA" if eng == "A" else "pD"
                return ps.tile([HIDDEN, 2 * CHUNK], fp32, tag=tag, bufs=2, name="pg")

            for s in range(n_slabs):
                xs = xs_tiles.pop(s)
                # ---- L0: K=32, 4-way row tiling; one quad = 4 concurrent MMs
                # spanning two 2-bank groups.
                for q in range(QUADS_PER_SLAB):
                    g0 = psum_group()
                    for i in range(2):
                        nc.tensor.matmul(
                            g0[:, i * CHUNK : (i + 1) * CHUNK],
                            w0s[32 * i : 32 * i + INPUT_DIM, :],
                            xs[32 * i : 32 * i + INPUT_DIM, q * CHUNK : (q + 1) * CHUNK],
                            start=True, stop=True,
                            tile_position=(32 * i, 0),
                        )
                    evac(hA[:, (4 * q) * CHUNK : (4 * q + 2) * CHUNK], g0[:, :], True)
                    g1 = psum_group()
                    for i in range(2, 4):
                        nc.tensor.matmul(
                            g1[:, (i - 2) * CHUNK : (i - 1) * CHUNK],
                            w0s[32 * i : 32 * i + INPUT_DIM, :],
                            xs[32 * i : 32 * i + INPUT_DIM, q * CHUNK : (q + 1) * CHUNK],
                            start=True, stop=True,
                            tile_position=(32 * i, 0),
                        )
                    evac(hA[:, (4 * q + 2) * CHUNK : (4 * q + 4) * CHUNK], g1[:, :], True)
                # prefetch input two slabs ahead (slabs 0/1 fetched up front)
                if s + 2 < n_slabs:
                    fetch_slab(s + 2)

                # ---- L1..L3: full-array matmuls, 2 chunks per PSUM group
                for (ws, hin, hout) in ((w1s, hA, hB), (w2s, hB, hA), (w3s, hA, hB)):
                    for t in range(GROUPS_PER_SLAB):
                        g = psum_group()
                        for k in range(2):
                            c = 2 * t + k
                            nc.tensor.matmul(
                                g[:, k * CHUNK : (k + 1) * CHUNK],
                                ws[:, :],
                                hin[:, c * CHUNK : (c + 1) * CHUNK],
                                start=True, stop=True,
                            )
                        evac(
                            hout[:, (2 * t) * CHUNK : (2 * t + 2) * CHUNK], g[:, :], True
                        )

                # ---- L4: M=16, 4-way col tiling packs 4 chunks/bank; a
                # 2-bank group holds 8 chunks of output.
                for gg in range(L4_GROUPS_PER_SLAB):
                    g = psum_group()
                    for b in range(2):
                        for j in range(4):
                            c = 8 * gg + 4 * b + j
                            nc.tensor.matmul(
                                g[32 * j : 32 * j + PADDED_OUT, b * CHUNK : (b + 1) * CHUNK],
                                w4s[:, :],
                                hB[:, c * CHUNK : (c + 1) * CHUNK],
                                start=True, stop=True,
                                tile_position=(0, 32 * j),
                            )
                    osb = io.tile(
                        [HIDDEN, 2 * CHUNK], fp16, tag="osb", bufs=4, name="osb"
                    )
                    evac(osb[:, :], g[:, :], False)
                    g_abs = s * L4_GROUPS_PER_SLAB + gg
                    for j in range(4):
                        nc.sync.dma_start(
                            out=yt[:, g_abs : g_abs + 1, :, j : j + 1, :],
                            in_=osb[32 * j : 32 * j + PADDED_OUT, :],
                        )
    _dedup_ldweights(nc)
    _split_waits(nc)
    return nc


def _split_weights(weights):
    ws = []
    off = 0
    ws.append(weights[off : off + HIDDEN * INPUT_DIM].reshape(HIDDEN, INPUT_DIM))
    off += HIDDEN * INPUT_DIM
    for _ in range(NUM_LAYERS - 1):
        ws.append(weights[off : off + HIDDEN * HIDDEN].reshape(HIDDEN, HIDDEN))
        off += HIDDEN * HIDDEN
    ws.append(weights[off : off + PADDED_OUT * HIDDEN].reshape(PADDED_OUT, HIDDEN))
    return ws


_NC_CACHE = {}


def make_in_maps(inputs: np.ndarray, weights: np.ndarray):
    ws = _split_weights(np.asarray(weights, dtype=np.float32))
    # stationary operands are lhsT = [K_in, M_out] = W.T; W0.T is stacked
    # four times for the four row-tiled strips.
    w0t = np.ascontiguousarray(ws[0].T).astype(np.float16)
    wmaps = {
        "w0": np.concatenate([w0t, w0t, w0t, w0t], axis=0),
        "w1": np.ascontiguousarray(ws[1].T).astype(np.float16),
        "w2": np.ascontiguousarray(ws[2].T).astype(np.float16),
        "w3": np.ascontiguousarray(ws[3].T).astype(np.float16),
        "w4": np.ascontiguousarray(ws[4].T).astype(np.float16),
    }
    in_maps = []
    for i in range(N_CORES):
        xc = inputs[i * B_CORE : (i + 1) * B_CORE]
        xtc = np.ascontiguousarray(xc.T).astype(np.float16)  # [32, B_CORE]
        # quad-strip layout: [128, B_CORE//4]
        xt4 = np.ascontiguousarray(
            xtc.reshape(INPUT_DIM, B_CORE // (4 * CHUNK), 4, CHUNK)
            .transpose(2, 0, 1, 3)
            .reshape(4 * INPUT_DIM, B_CORE // 4)
        )
        in_maps.append({"xt": xt4, **wmaps})
    return in_maps


def kernel(inputs: np.ndarray, weights: np.ndarray) -> np.ndarray:
    from concourse.bass_utils import run_bass_kernel_spmd

    assert inputs.shape == (B, INPUT_DIM), inputs.shape
    in_maps = make_in_maps(inputs, weights)
    if "nc" not in _NC_CACHE:
        _NC_CACHE["nc"] = build()
    nc = _NC_CACHE["nc"]
    res = run_bass_kernel_spmd(nc, in_maps, list(range(N_CORES)))
    outs = [
        np.ascontiguousarray(r["yt"].reshape(PADDED_OUT, B_CORE).T.astype(np.float32))
        for r in res.results
    ]
    return np.concatenate(outs, axis=0)[:, :OUTPUT_DIM]


# revision 12
# speedup vs baseline: 1.2518x; 1.0428x over previous
"""Trainium2 Bass kernel for nn_FFMLP (4-layer MLP, hidden=128, relu).

Strategy (pure data parallel, batch sharded 8 ways):
- Feature-major on-chip layout: activations live as [feat, batch]; each layer
  is a K<=128 matmul with the tiny replicated weight stationary and the
  activation stream moving. fp16 operands, fp32 PSUM.
- Layer-batched slab schedule: per slab of 32 chunks, run all of L0, then all
  of L1, ... so consecutive PE matmuls share weights (one LdWeights per layer
  per slab instead of one per matmul -- the interleaved baseline paid ~50us
  of LDWEIGHTS on the PE queue).
- L0 (K=32) packs 4 chunks into concurrent row-tiled matmuls at
  tile_position (32i, 0); L4 (M=16) packs 4 chunks per PSUM bank via column
  tiling (0, 32j). Tiled matmuls execute concurrently on the PE.
- PSUM: 8 banks as two double-buffered 2-bank groups, one owned by the
  Scalar (ACT) engine, one by Vector (DVE). PSUM->SBUF relu evacuation is
  the structural bottleneck (only these two engines can read PSUM);
  a 6:5 ACT:DVE group pattern balances their 0.83 vs 1.10 ns/col rates and
  keeps both ~100% busy without bank starvation.
- Output is packed fp16 in a (group, bank, strip) lexicographic layout so the
  host just reshapes; final cast to fp32 on host.
"""
import sys

if "/opt/trn_rl_repo" not in sys.path:
    sys.path.insert(0, "/opt/trn_rl_repo")

import numpy as np

import concourse.bass as bass
import concourse.mybir as mybir
import concourse.tile as tile

INPUT_DIM = 32
OUTPUT_DIM = 16
HIDDEN = 128
PADDED_OUT = 16
NUM_LAYERS = 4
B = 524288
N_CORES = 8
B_CORE = B // N_CORES  # 65536
CHUNK = 512
N_CHUNKS = B_CORE // CHUNK  # 128
SLAB = 32  # chunks per slab (layer-batched unit)
N_SLABS = N_CHUNKS // SLAB  # 4
QUADS_PER_SLAB = SLAB // 4  # 8 (4-chunk quads for L0 row tiling)
GROUPS_PER_SLAB = SLAB // 2  # 16 2-chunk PSUM groups per layer phase
L4_GROUPS_PER_SLAB = SLAB // 8  # 4 (8 chunks of output per 2-bank group)
N_L4_GROUPS = N_CHUNKS // 8  # 16

fp16 = mybir.dt.float16
fp32 = mybir.dt.float32
RELU = mybir.ActivationFunctionType.Relu

# evac engine pattern: 6 ACT : 5 DVE matches the measured ~1020 : ~1205 ns
# per-op busy times (both engines' spans balance at ~157us)
EVAC_PAT = "ADADADADADA"


def _split_waits(nc, max_waits=1):
    """walrus in this image rejects >1 semaphore wait per instruction on some
    formats; split excess waits onto preceding NOPs on the same engine queue
    (queues are in-order, so semantics are preserved)."""
    n_new = 0
    for bb in nc.main_func.blocks:
        out_list = []
        changed = False
        for ins in bb.instructions:
            si = ins.sync_info
            if si is not None and si.on_wait and len(si.on_wait) > max_waits:
                waits = list(si.on_wait)
                extra, keep = waits[:-max_waits], waits[-max_waits:]
                while extra:
                    chunk, extra = extra[:max_waits], extra[max_waits:]
                    n_new += 1
                    nop = mybir.InstNoOp(name=f"I-waitsplit-{n_new}", ins=[], outs=[])
                    nop.engine = ins.engine
                    nop.sync_info = mybir.SyncInfo(on_wait=chunk, on_update=[])
                    out_list.append(nop)
                ins.sync_info = mybir.SyncInfo(on_wait=keep, on_update=si.on_update)
                changed = True
            out_list.append(ins)
        if changed:
            bb.instructions = out_list
    return n_new


def _ldw_rect(ins):
    """PE-array rectangle (r0, r1, c0, c1) occupied by an InstLdweights."""
    tp = ins.tile_position
    ts = getattr(ins, "tile_size", None)
    r0, c0 = (tp if tp else (0, 0))
    if ts:
        rows, cols = ts
    else:
        rows, cols = 128, 128
    return (r0, r0 + rows, c0, c0 + cols)


def _dedup_ldweights(nc):
    """Tile emits an explicit InstLdweights before every matmul. Weights at a
    given tile rectangle stay resident until an overlapping load clobbers
    them, so replace reloads of already-resident weights with NOPs (keeping
    sync_info). Tracks residency per array rectangle, which handles the
    alternating tile positions of the row/col-tiled L0/L4 phases."""
    n = 0
    for bb in nc.main_func.blocks:
        il = list(bb.instructions)
        live = {}  # rect -> content key
        changed = False
        for idx, ins in enumerate(il):
            if ins.engine != mybir.EngineType.PE:
                continue
            if isinstance(ins, mybir.InstLdweights):
                rect = _ldw_rect(ins)
                key = (
                    repr(ins.ins[0]),
                    str(ins.tile_position),
                    str(getattr(ins, "tile_size", None)),
                    str(ins.perf_mode),
                    bool(ins.is_transpose),
                )
                if live.get(rect) == key:
                    nop = mybir.InstNoOp(name=ins.name, ins=[], outs=[])
                    nop.engine = ins.engine
                    nop.sync_info = ins.sync_info
                    il[idx] = nop
                    changed = True
                    n += 1
                else:
                    r0, r1, c0, c1 = rect
                    for other in list(live):
                        o0, o1, p0, p1 = other
                        if r0 < o1 and o0 < r1 and c0 < p1 and p0 < c1:
                            del live[other]
                    live[rect] = key
        if changed:
            bb.instructions = il
    return n


def build(n_slabs=N_SLABS):
    nc = bass.Bass()
    n_chunks = n_slabs * SLAB
    # xt4: quad-strip layout -- xt4[32*i + f, q*CHUNK + c] = x.T[f, (4q+i)*CHUNK + c]
    # so each quad of 4 chunks feeds 4 concurrent row-tiled K=32 L0 matmuls.
    xt = nc.declare_dram_parameter(
        "xt", [4 * INPUT_DIM, n_chunks * CHUNK // 4], fp16, isOutput=False
    )
    wd = nc.declare_dram_parameter(
        "wd", [HIDDEN, 4 * HIDDEN + PADDED_OUT], fp16, isOutput=False
    )
    # yt[o, g, j, b, c] = y.T[o, (8g + 4b + j)*CHUNK + c] -- the (j, b) order
    # lets one rearranged DMA per L4 group write all 4 partition strips;
    # host transposes (0,1,3,2,4) and reshapes to [16, B_CORE].
    n_l4_groups = n_chunks // 8
    yt = nc.declare_dram_parameter(
        "yt", [PADDED_OUT, n_l4_groups, 4, 2, CHUNK], fp16, isOutput=True
    )

    with tile.TileContext(nc) as tc:
        with (
            tc.tile_pool(name="wp", bufs=1) as wp,
            tc.tile_pool(name="io", bufs=1) as io,
            tc.tile_pool(name="hp", bufs=1) as hp,
            tc.tile_pool(name="ps", bufs=1, space="PSUM") as ps,
        ):
            wall = wp.tile(
                [HIDDEN, 4 * HIDDEN + PADDED_OUT], fp16, tag="wall", name="wall"
            )
            nc.sync.dma_start(out=wall, in_=wd[:, :])
            w0s = wall[:, 0:HIDDEN]
            w1s = wall[:, HIDDEN : 2 * HIDDEN]
            w2s = wall[:, 2 * HIDDEN : 3 * HIDDEN]
            w3s = wall[:, 3 * HIDDEN : 4 * HIDDEN]
            w4s = wall[:, 4 * HIDDEN : 4 * HIDDEN + PADDED_OUT]

            # hidden activation ping-pong buffers, one slab each
            hA = hp.tile([HIDDEN, SLAB * CHUNK], fp16, tag="hA", name="hA")
            hB = hp.tile([HIDDEN, SLAB * CHUNK], fp16, tag="hB", name="hB")

            SLAB_COLS = SLAB * CHUNK // 4  # xt cols per slab (quad-packed)

            xs_tiles = {}

            def fetch_slab(s):
                if s >= n_slabs:
                    return
                xs = io.tile(
                    [4 * INPUT_DIM, SLAB_COLS], fp16, tag="xin", bufs=2, name="xs"
                )
                # two half-DMAs so L0 of the first quads can start before the
                # whole slab lands (subtile deps gate on each DMA separately)
                h = SLAB_COLS // 2
                nc.sync.dma_start(
                    out=xs[:, 0:h], in_=xt[:, s * SLAB_COLS : s * SLAB_COLS + h]
                )
                nc.sync.dma_start(
                    out=xs[:, h:SLAB_COLS],
                    in_=xt[:, s * SLAB_COLS + h : (s + 1) * SLAB_COLS],
                )
                xs_tiles[s] = xs

            # HAM warm-up: dummy matmuls keep the PE busy while the first
            # input slab lands, so real matmuls start at 2.4 GHz instead of
            # paying the ~3.4us cold window at 1.2 GHz.
            fetch_slab(0)
            pwarm = ps.tile([HIDDEN, 2 * CHUNK], fp32, tag="pA", bufs=2, name="pwarm")
            for _ in range(12):
                nc.tensor.matmul(
                    pwarm[:, 0:HIDDEN], w1s[:, :], w2s[:, 0:HIDDEN],
                    start=True, stop=True,
                )
            fetch_slab(1)

            evac_state = {"i": 0}

            def evac(dst, src, relu):
                eng = EVAC_PAT[evac_state["i"] % len(EVAC_PAT)]
                evac_state["i"] += 1
                if eng == "A":
                    if relu:
                        nc.scalar.activation(dst, src, RELU)
                    else:
                        nc.scalar.copy(out=dst, in_=src)
                else:
                    if relu:
                        nc.vector.tensor_scalar_max(dst, src, 0.0)
                    else:
                        nc.vector.tensor_copy(dst, src)

            def psum_group():
                eng = EVAC_PAT[evac_state["i"] % len(EVAC_PAT)]
                tag = "pA" if eng == "A" else "pD"
                return ps.tile([HIDDEN, 2 * CHUNK], fp32, tag=tag, bufs=2, name="pg")

            for s in range(n_slabs):
                xs = xs_tiles.pop(s)
                # ---- L0: K=32, 4-way row tiling; one quad = 4 concurrent MMs
                # spanning two 2-bank groups.
                for q in range(QUADS_PER_SLAB):
                    g0 = psum_group()
                    for i in range(2):
                        nc.tensor.matmul(
                            g0[:, i * CHUNK : (i + 1) * CHUNK],
                            w0s[32 * i : 32 * i + INPUT_DIM, :],
                            xs[32 * i : 32 * i + INPUT_DIM, q * CHUNK : (q + 1) * CHUNK],
                            start=True, stop=True,
                            tile_position=(32 * i, 0),
                        )
                    evac(hA[:, (4 * q) * CHUNK : (4 * q + 2) * CHUNK], g0[:, :], True)
                    g1 = psum_group()
                    for i in range(2, 4):
                        nc.tensor.matmul(
                            g1[:, (i - 2) * CHUNK : (i - 1) * CHUNK],
                            w0s[32 * i : 32 * i + INPUT_DIM, :],
                            xs[32 * i : 32 * i + INPUT_DIM, q * CHUNK : (q + 1) * CHUNK],
                            start=True, stop=True,
                            tile_position=(32 * i, 0),
                        )
                    evac(hA[:, (4 * q + 2) * CHUNK : (4 * q + 4) * CHUNK], g1[:, :], True)
                # prefetch input two slabs ahead (slabs 0/1 fetched up front)
                if s + 2 < n_slabs:
                    fetch_slab(s + 2)

                # ---- L1..L3: full-array matmuls, 2 chunks per PSUM group
                for (ws, hin, hout) in ((w1s, hA, hB), (w2s, hB, hA), (w3s, hA, hB)):
                    for t in range(GROUPS_PER_SLAB):
                        g = psum_group()
                        for k in range(2):
                            c = 2 * t + k
                            nc.tensor.matmul(
                                g[:, k * CHUNK : (k + 1) * CHUNK],
                                ws[:, :],
                                hin[:, c * CHUNK : (c + 1) * CHUNK],
                                start=True, stop=True,
                            )
                        evac(
                            hout[:, (2 * t) * CHUNK : (2 * t + 2) * CHUNK], g[:, :], True
                        )

                # ---- L4: M=16, 4-way col tiling packs 4 chunks/bank; a
                # 2-bank group holds 8 chunks of output.
                for gg in range(L4_GROUPS_PER_SLAB):
                    g = psum_group()
                    for b in range(2):
                        for j in range(4):
                            c = 8 * gg + 4 * b + j
                            nc.tensor.matmul(
                                g[32 * j : 32 * j + PADDED_OUT, b * CHUNK : (b + 1) * CHUNK],
                                w4s[:, :],
                                hB[:, c * CHUNK : (c + 1) * CHUNK],
                                start=True, stop=True,
                                tile_position=(0, 32 * j),
                            )
                    osb = io.tile(
                        [HIDDEN, 2 * CHUNK], fp16, tag="osb", bufs=4, name="osb"
                    )
                    evac(osb[:, :], g[:, :], False)
                    g_abs = s * L4_GROUPS_PER_SLAB + gg
                    for j in range(4):
                        eng = nc.sync if j < 2 else nc.gpsimd
                        eng.dma_start(
                            out=yt[:, g_abs : g_abs + 1, j : j + 1, :, :],
                            in_=osb[32 * j : 32 * j + PADDED_OUT, :],
                        )
    _dedup_ldweights(nc)
    _split_waits(nc)
    return nc


def _split_weights(weights):
    ws = []
    off = 0
    ws.append(weights[off : off + HIDDEN * INPUT_DIM].reshape(HIDDEN, INPUT_DIM))
    off += HIDDEN * INPUT_DIM
    for _ in range(NUM_LAYERS - 1):
        ws.append(weights[off : off + HIDDEN * HIDDEN].reshape(HIDDEN, HIDDEN))
        off += HIDDEN * HIDDEN
    ws.append(weights[off : off + PADDED_OUT * HIDDEN].reshape(PADDED_OUT, HIDDEN))
    return ws


_NC_CACHE = {}


def make_in_maps(inputs: np.ndarray, weights: np.ndarray):
    ws = _split_weights(np.asarray(weights, dtype=np.float32))
    # stationary operands are lhsT = [K_in, M_out] = W.T; W0.T is stacked
    # four times for the four row-tiled strips.
    w0t = np.ascontiguousarray(ws[0].T).astype(np.float16)
    wd = np.concatenate(
        [
            np.concatenate([w0t, w0t, w0t, w0t], axis=0),  # [128, 128]
            np.ascontiguousarray(ws[1].T).astype(np.float16),
            np.ascontiguousarray(ws[2].T).astype(np.float16),
            np.ascontiguousarray(ws[3].T).astype(np.float16),
            np.ascontiguousarray(ws[4].T).astype(np.float16),  # [128, 16]
        ],
        axis=1,
    )
    wmaps = {"wd": np.ascontiguousarray(wd)}
    in_maps = []
    for i in range(N_CORES):
        xc = inputs[i * B_CORE : (i + 1) * B_CORE]
        xtc = np.ascontiguousarray(xc.T).astype(np.float16)  # [32, B_CORE]
        # quad-strip layout: [128, B_CORE//4]
        xt4 = np.ascontiguousarray(
            xtc.reshape(INPUT_DIM, B_CORE // (4 * CHUNK), 4, CHUNK)
            .transpose(2, 0, 1, 3)
            .reshape(4 * INPUT_DIM, B_CORE // 4)
        )
        in_maps.append({"xt": xt4, **wmaps})
    return in_maps


def kernel(inputs: np.ndarray, weights: np.ndarray) -> np.ndarray:
    from concourse.bass_utils import run_bass_kernel_spmd

    assert inputs.shape == (B, INPUT_DIM), inputs.shape
    in_maps = make_in_maps(inputs, weights)
    if "nc" not in _NC_CACHE:
        _NC_CACHE["nc"] = build()
    nc = _NC_CACHE["nc"]
    res = run_bass_kernel_spmd(nc, in_maps, list(range(N_CORES)))
    outs = [
        np.ascontiguousarray(
            r["yt"]
            .transpose(0, 1, 3, 2, 4)  # (o, g, j, b, c) -> (o, g, b, j, c)
            .reshape(PADDED_OUT, B_CORE)
            .T.astype(np.float32)
        )
        for r in res.results
    ]
    return np.concatenate(outs, axis=0)[:, :OUTPUT_DIM]
